# revision 13
# baseline (speedup 1.0000x reference)
"""Cross-attention kernel for 8 TRN2 NeuronCores.

Problem: B=4, T_V=8192, T_T=77, C=1024, H=16, D=64 (f32).
  q = video @ Wq.T ; k,v = text @ W.T ; out = softmax(qk/sqrt(D)) v @ Wo.T

Sharding: data-parallel over (batch, T_V/2) -> 8 shards of [4096, 1024].
Each core gets its video shard, its batch's text, and all weights.
No collectives.

On-chip dataflow ("transposed": rows of video on the FREE dim), all
matmul operands bf16 (host-cast), PSUM f32 for accumulating GEMMs:
  Q^T = WqT-chunks . X^T          (bf16 matmuls, N=512)
  K^T [C, T] from text, V natural [T, C] with an appended ones column
  per head h: S^T = K_h^T . Q_h^T -> exp on ScalarE (scale=1/8 folded
  in, no max-subtraction: scores are O(1) bounded) -> bf16 es
  O'^T = [V_h|1] . es^T  (bf16 PSUM out; row 64 = softmax denominator)
  per head pair: reciprocal rows into a [2,512] tile, one DMA
  partition-broadcast -> rb [128,512], two DVE multiplies (2x mode,
  all-bf16) write the normalized O^T chunk.
  out = O^T-chunks . WoT in natural [m, n] layout -> SBUF -> DRAM f32.
"""

import sys

if "/opt/trn_rl_repo" not in sys.path:
    sys.path.insert(0, "/opt/trn_rl_repo")

import ml_dtypes
import numpy as np

import concourse.bacc as bacc
import concourse.mybir as mybir
import concourse.tile as tile
from concourse.bass_utils import run_bass_kernel_spmd

F32 = mybir.dt.float32
BF16 = mybir.dt.bfloat16
AF = mybir.ActivationFunctionType
MULT = mybir.AluOpType.mult

B, T_V, T_T, C, H = 4, 8192, 77, 1024, 16
D = C // H            # 64
P = 128
KC = C // P           # 8 contraction chunks
M = T_V // 2          # 4096 rows per core
MB = 512              # m-block (rows processed per pipeline stage)
NBLK = M // MB        # 8
MSUB = MB // P        # 4 output row-chunks per block
T = T_T               # 77
TP = 80               # padded T
SCALE = 1.0 / float(np.sqrt(D))

_CACHED_NC = None


def _build(repeat: int = 1):
    nc = bacc.Bacc(name="cross_attention")

    # xt pre-chunked on host: [NBLK, C, MB] so each block DMA is contiguous
    xt = nc.dram_tensor("xt", [NBLK, C, MB], BF16, kind="ExternalInput")
    yt = nc.dram_tensor("yt", [C, T], BF16, kind="ExternalInput")
    wqt = nc.dram_tensor("wqt", [C, C], BF16, kind="ExternalInput")
    wkt = nc.dram_tensor("wkt", [C, C], BF16, kind="ExternalInput")
    wvt = nc.dram_tensor("wvt", [C, C], BF16, kind="ExternalInput")
    wot = nc.dram_tensor("wot", [C, C], BF16, kind="ExternalInput")
    out = nc.dram_tensor("out", [M, C], F32, kind="ExternalOutput")

    xt_v = xt[:, :, :].rearrange("j (kc p) m -> p j kc m", p=P)
    yt_v = yt[:, :].rearrange("(kc p) t -> p kc t", p=P)
    wq_v = wqt[:, :].rearrange("(kc p) n -> p kc n", p=P)
    wk_v = wkt[:, :].rearrange("(kc p) n -> p kc n", p=P)
    wv_v = wvt[:, :].rearrange("(kc p) n -> p kc n", p=P)
    wo_v = wot[:, :].rearrange("(kc p) n -> p kc n", p=P)

    with tile.TileContext(nc) as tc:
        with (
            tc.tile_pool(name="wq", bufs=1) as wq_pool,
            tc.tile_pool(name="wo", bufs=1) as wo_pool,
            tc.tile_pool(name="kt", bufs=1) as kt_pool,
            tc.tile_pool(name="vv", bufs=1) as v_pool,
        ):
            wq_sb = wq_pool.tile([P, KC, C], BF16)
            wo_sb = wo_pool.tile([P, KC, C], BF16)
            kt_sb = kt_pool.tile([P, KC, TP], BF16)
            v_sb = v_pool.tile([T, H, D + 1], BF16)

            # ---- prologue: K^T and V from text; load Wq/Wo ----
            # Wq loads FIRST so block-0 Q-proj isn't starved behind the
            # K/V weight traffic.
            for kc in range(KC):
                nc.sync.dma_start(wq_sb[:, kc, :], wq_v[:, kc, :])
            with (
                tc.tile_pool(name="wkv", bufs=1) as wkv_pool,
                tc.tile_pool(name="yt", bufs=1) as yt_pool,
                tc.tile_pool(name="pspro", bufs=2, space="PSUM") as ps_pro,
            ):
                yt_sb = yt_pool.tile([P, KC, TP], BF16)
                nc.vector.memset(yt_sb[:], 0.0)
                nc.sync.dma_start(yt_sb[:, :, :T], yt_v[:])

                wk_sb = wkv_pool.tile([P, KC, C], BF16, tag="wkv")
                for kc in range(KC):
                    nc.sync.dma_start(wk_sb[:, kc, :], wk_v[:, kc, :])
                # K^T [C, T]: chunk nc_ holds rows 128*nc_..128*nc_+128
                for nc_ in range(KC):
                    psk_full = ps_pro.tile([P, MB], F32, tag="pro", name="psk")
                    psk = psk_full[:, :TP]
                    for kc in range(KC):
                        nc.tensor.matmul(
                            psk[:],
                            wk_sb[:, kc, nc_ * P : (nc_ + 1) * P],
                            yt_sb[:, kc, :],
                            start=(kc == 0),
                            stop=(kc == KC - 1),
                        )
                    nc.vector.tensor_copy(out=kt_sb[:, nc_, :T], in_=psk[:, :T])

                wv_sb = wkv_pool.tile([P, KC, C], BF16, tag="wkv")
                for kc in range(KC):
                    nc.sync.dma_start(wv_sb[:, kc, :], wv_v[:, kc, :])
                # V natural [T, C] written per 512-wide column slab into
                # the strided per-head layout v_sb[t, h, 0:64]
                for half in range(2):
                    psv_full = ps_pro.tile([P, MB], F32, tag="pro", name="psv")
                    psv = psv_full[:T, :]
                    for kc in range(KC):
                        nc.tensor.matmul(
                            psv[:],
                            yt_sb[:, kc, :T],
                            wv_sb[:, kc, half * MB : (half + 1) * MB],
                            start=(kc == 0),
                            stop=(kc == KC - 1),
                        )
                    nc.vector.tensor_copy(
                        out=v_sb[:, half * 8 : (half + 1) * 8, 0:D],
                        in_=psv[:].rearrange("t (h d) -> t h d", d=D),
                    )
                nc.vector.memset(v_sb[:, :, D : D + 1], 1.0)

            for kc in range(KC):
                nc.sync.dma_start(wo_sb[:, kc, :], wo_v[:, kc, :])

            # ---- main pipeline over m-blocks ----
            with (
                tc.tile_pool(name="xt", bufs=3) as xt_pool,
                tc.tile_pool(name="qt", bufs=2) as qt_pool,
                tc.tile_pool(name="ot", bufs=2) as ot_pool,
                tc.tile_pool(name="es", bufs=4) as es_pool,
                tc.tile_pool(name="rc", bufs=4) as rc_pool,
                tc.tile_pool(name="rb", bufs=4) as rb_pool,
                tc.tile_pool(name="ob", bufs=4) as ob_pool,
                tc.tile_pool(name="psq", bufs=2, space="PSUM") as ps_q,
                tc.tile_pool(name="pss", bufs=2, space="PSUM") as ps_s,
                tc.tile_pool(name="pso", bufs=2, space="PSUM") as ps_o,
                tc.tile_pool(name="psout", bufs=2, space="PSUM") as ps_out,
            ):
                def emit_out_tile(j, ot_t, mi, nh):
                    # one [128, 512] tile of the output projection,
                    # natural [m, n] layout
                    pst = ps_out.tile([P, MB], F32, tag="psout")
                    for cc in range(KC):
                        nc.tensor.matmul(
                            pst[:],
                            ot_t[:, cc, mi * P : (mi + 1) * P],
                            wo_sb[:, cc, nh * MB : (nh + 1) * MB],
                            start=(cc == 0),
                            stop=(cc == KC - 1),
                        )
                    ob = ob_pool.tile([P, MB], F32, tag="ob")
                    nc.any.tensor_copy(out=ob[:], in_=pst[:])
                    nc.sync.dma_start(
                        out[
                            j * MB + mi * P : j * MB + (mi + 1) * P,
                            nh * MB : (nh + 1) * MB,
                        ],
                        ob[:],
                    )

                def emit_out_proj(j, ot_t):
                    for mi in range(MSUB):
                        for nh in range(2):
                            emit_out_tile(j, ot_t, mi, nh)

                pending = None
                for j in [jj for _ in range(repeat) for jj in range(NBLK)]:
                    xt_t = xt_pool.tile([P, KC, MB], BF16, tag="xt")
                    nc.sync.dma_start(xt_t[:], xt_v[:, j, :, :])

                    # Q^T chunks for this block
                    qt_t = qt_pool.tile([P, KC, MB], BF16, tag="qt")
                    for nc_ in range(KC):
                        psq = ps_q.tile([P, MB], F32, tag="psq")
                        for kc in range(KC):
                            nc.tensor.matmul(
                                psq[:],
                                wq_sb[:, kc, nc_ * P : (nc_ + 1) * P],
                                xt_t[:, kc, :],
                                start=(kc == 0),
                                stop=(kc == KC - 1),
                            )
                        nc.any.tensor_copy(out=qt_t[:, nc_, :], in_=psq[:])

                    # attention heads, processed in pairs sharing a
                    # reciprocal tile + one broadcast DMA. The two S
                    # matmuls contract disjoint row-groups (0-63 /
                    # 64-127) so tile_position lets them overlap in the
                    # PE array.
                    ot_t = ot_pool.tile([P, KC, MB], BF16, tag="ot")

                    for jc in range(KC):
                        rc2 = rc_pool.tile([33, MB], BF16, tag="rc")
                        psss = []
                        for hf in range(2):
                            lo, hi = 64 * hf, 64 * hf + 64
                            pss = ps_s.tile([T, MB], F32, tag="pss")
                            nc.tensor.matmul(
                                pss[:],
                                kt_sb[lo:hi, jc, :T],
                                qt_t[lo:hi, jc, :],
                                start=True,
                                stop=True,
                                tile_position=(64 * hf, 0),
                            )
                            psss.append(pss)
                        ess = []
                        for hf in range(2):
                            es = es_pool.tile([T, MB], BF16, tag="es")
                            nc.scalar.activation(
                                es[:], psss[hf][:], AF.Exp, scale=SCALE
                            )
                            ess.append(es)
                        psos = []
                        for hf in range(2):
                            h = 2 * jc + hf
                            pso = ps_o.tile([D + 1, MB], F32, tag="pso")
                            nc.tensor.matmul(
                                pso[:], v_sb[:, h, :], ess[hf][:],
                                start=True, stop=True,
                            )
                            psos.append(pso)
                            # softmax denominator lives in PSUM row D
                            with nc.allow_low_precision(
                                reason="bf16 recip of softmax denom"
                            ):
                                nc.vector.reciprocal(
                                    rc2[32 * hf : 32 * hf + 1, :],
                                    pso[D : D + 1, :],
                                )
                        rb = rb_pool.tile([P, MB], BF16, tag="rb")
                        nc.sync.dma_start(
                            rb[:],
                            rc2[0:33:32, None, :].to_broadcast(
                                (2, D, MB)
                            ),
                        )
                        for hf in range(2):
                            lo, hi = 64 * hf, 64 * hf + 64
                            nc.vector.tensor_tensor(
                                ot_t[lo:hi, jc, :],
                                psos[hf][0:D, :],
                                rb[lo:hi, :],
                                MULT,
                            )

                        # software pipeline: one output-projection tile
                        # of the PREVIOUS block per head-pair slot, so
                        # the PE fills normalize-chain gaps instead of
                        # stalling.
                        if pending is not None:
                            emit_out_tile(
                                pending[0], pending[1], jc // 2, jc % 2
                            )

                    pending = (j, ot_t)
                emit_out_proj(*pending)
    nc.finalize()
    return nc


def _get_nc(repeat: int = 1):
    global _CACHED_NC
    if _CACHED_NC is None:
        _CACHED_NC = {}
    if repeat not in _CACHED_NC:
        _CACHED_NC[repeat] = _build(repeat)
    return _CACHED_NC[repeat]


def make_in_maps(video_features, text_features, Wq, Wk, Wv, Wo):
    bf = ml_dtypes.bfloat16
    wqt = np.ascontiguousarray(np.asarray(Wq, dtype=np.float32).T).astype(bf)
    wkt = np.ascontiguousarray(np.asarray(Wk, dtype=np.float32).T).astype(bf)
    wvt = np.ascontiguousarray(np.asarray(Wv, dtype=np.float32).T).astype(bf)
    wot = np.ascontiguousarray(np.asarray(Wo, dtype=np.float32).T).astype(bf)
    video_features = np.asarray(video_features, dtype=np.float32)
    text_features = np.asarray(text_features, dtype=np.float32)

    in_maps = []
    for c in range(8):
        b, half = divmod(c, 2)
        xs = video_features[b, half * M : (half + 1) * M, :]  # [M, C]
        xtb = np.ascontiguousarray(xs.T).astype(bf)           # [C, M]
        # pre-chunk: [NBLK, C, MB] so per-block DMA reads are contiguous
        xtb = np.ascontiguousarray(
            xtb.reshape(C, NBLK, MB).transpose(1, 0, 2)
        )
        in_maps.append(
            {
                "xt": xtb,
                "yt": np.ascontiguousarray(text_features[b].T).astype(bf),
                "wqt": wqt,
                "wkt": wkt,
                "wvt": wvt,
                "wot": wot,
            }
        )
    return in_maps


def kernel(video_features, text_features, Wq, Wk, Wv, Wo, **_unused):
    in_maps = make_in_maps(video_features, text_features, Wq, Wk, Wv, Wo)
    res = run_bass_kernel_spmd(_get_nc(), in_maps, core_ids=list(range(8)))
    outf = np.empty((B, T_V, C), dtype=np.float32)
    for c in range(8):
        b, half = divmod(c, 2)
        outf[b, half * M : (half + 1) * M, :] = res.results[c]["out"]
    return outf
